# revision 32
# baseline (speedup 1.0000x reference)
"""Causal self-attention (RoPE, GQA) on 8 Trainium2 NeuronCores.

Sharding: 2-way data-parallel over batch x 4-way tensor-parallel over heads.
Core c handles batch c//4 and head-group c%4 (4 q-heads, 2 kv-heads).
Each core computes its partial output projection (wo row-shard); the host
sums the 4 partials per batch (the "all-reduce" happens in the unshard step).

Layouts (all transposed so no on-device transposes are ever needed):
  - x is fed as xT [D, S] in bf16; Q/K are produced as [head_dim, S] via
    lhsT=weight-slice, rhs=xT; V is produced as [S, dv] via lhsT=xT-slice.
  - RoPE: weight rows are pre-permuted (even components -> rows 0:64,
    odd -> rows 64:128) so rotation is elementwise on row halves.
  - scores are computed transposed [k, q] so softmax numerator feeds the
    PV matmul directly; Z (denominator) via a ones-vector matmul; the
    normalization 1/Z via reciprocal on DVE plus a K=1 broadcast matmul.

PSUM plan (8 banks): scores 2-bank tiles double-buffered (4), O^T
accumulator double-buffered (2), a 2-bank ring shared by the Z accumulator
and the 1/Z broadcast (2). The output projection is deferred one q-chunk
so its inputs are always long-ready, and its PSUM evac alternates between
the ACT and DVE engines to keep ACT free for the exp stream.

Scheduling constraint honored throughout: a DVE TensorTensor can carry at
most ONE sync-wait, so every TT here has at most one freshly-produced
cross-engine operand.
"""

import sys
import numpy as np
import ml_dtypes

sys.path.insert(0, "/opt/trn_rl_repo")

import concourse.bass as bass
import concourse.bacc as bacc
import concourse.mybir as mybir
from concourse import tile
from concourse.bass_utils import run_bass_kernel_spmd

F32 = mybir.dt.float32
F32R = mybir.dt.float32r
BF16 = mybir.dt.bfloat16
AF = mybir.ActivationFunctionType
OP = mybir.AluOpType

B, S, D = 2, 2048, 2048
HQ, HKV, HD = 16, 8, 128
ROPE_THETA = 10000.0
NCORES, TP = 8, 4
HQL, HKL = HQ // TP, HKV // TP        # 4 q heads, 2 kv heads per core
NKT = D // 128                        # 16 contraction tiles
QC = 512                              # q-chunk width
NQC = S // QC                         # 4 q chunks
NSB = S // 128                        # 16 s-blocks
SCALE = 1.0 / float(np.sqrt(HD))
BF = ml_dtypes.bfloat16


def _build_nc():
    nc = bacc.Bacc("TRN2", target_bir_lowering=False)

    # p-major layouts: every DMA moves >=4KB contiguous per partition row
    # (1KB rows are descriptor-rate-bound at ~100ns/packet on the HWDGE)
    xT_d = nc.dram_tensor("xT", [128, NKT, S], BF16, kind="ExternalInput")
    wq_d = nc.dram_tensor("wq_t", [128, NKT, HQL * HD], BF16, kind="ExternalInput")
    wk_d = nc.dram_tensor("wk_t", [128, NKT, HKL * HD], BF16, kind="ExternalInput")
    wv_d = nc.dram_tensor("wv_t", [128, NKT, HKL * HD], BF16, kind="ExternalInput")
    wo_d = nc.dram_tensor("wo_t", [128, HQL, D], BF16, kind="ExternalInput")
    cos_d = nc.dram_tensor("cos128", [128, S], BF16, kind="ExternalInput")
    sin_d = nc.dram_tensor("sinM", [128, S], BF16, kind="ExternalInput")
    mask_d = nc.dram_tensor("dmask", [2, 128, 2 * QC], BF16, kind="ExternalInput")
    out_d = nc.dram_tensor("out", [NSB, 128, D], BF16, kind="ExternalOutput")

    with tile.TileContext(nc) as tc:
        with (
            tc.tile_pool(name="resident", bufs=1) as res,
            tc.tile_pool(name="ropetmp", bufs=2) as rtmp,
            tc.tile_pool(name="epool", bufs=3) as epool,
            tc.tile_pool(name="small", bufs=2) as small,
            tc.tile_pool(name="outp", bufs=2) as outp,
        ):
            # ---------- resident tiles: x and weights live in SBUF whole;
            # DMAs are chunked so the first matmuls start after ~2us ----------
            x_sb = res.tile([128, NKT, S], BF16)
            wq_sb = res.tile([128, NKT, HQL * HD], BF16)
            wk_sb = res.tile([128, NKT, HKL * HD], BF16)
            wv_sb = res.tile([128, NKT, HKL * HD], BF16)
            wo_sb = res.tile([128, HQL, D], BF16)
            cos_sb = res.tile([128, S], BF16)
            sin_sb = res.tile([128, S], BF16)
            mask_sb = res.tile([128, 2, 2 * QC], BF16)

            ones_bf = res.tile([128, 1], BF16)
            nc.vector.memset(ones_bf[:], 1.0)
            ones_rf = res.tile([1, 128], F32)
            nc.vector.memset(ones_rf[:], 1.0)
            ones_r = res.tile([1, 128], F32R)
            nc.vector.tensor_copy(ones_r[:], ones_rf[:])

            # outputs of phase 1 (resident through phase 2/3)
            qt_sb = [res.tile([128, S], BF16, tag=f"qt{h}", name=f"qt{h}") for h in range(HQL)]
            kt_sb = [res.tile([128, S], BF16, tag=f"kt{h}", name=f"kt{h}") for h in range(HKL)]
            v_sb = res.tile([128, NSB, HKL * HD], BF16)
            ot_sb = [res.tile([128, S], BF16, tag=f"ot{h}", name=f"ot{h}") for h in range(HQL)]

            # ---------- input DMAs: x per k-tile on the sync queue (4KB
            # rows), weights in 4-ktile chunks on the scalar queue ----------
            # first 4 k-tiles individually (earliest availability), the rest
            # in 4-ktile chunks: 16KB contiguous per partition row keeps the
            # HWDGE descriptor count low (the real DMA-rate limiter)
            for kt in range(4):
                nc.sync.dma_start(x_sb[:, kt, :], xT_d[:, kt, :])
            for c in range(1, 4):
                nc.sync.dma_start(x_sb[:, 4 * c:4 * c + 4, :],
                                  xT_d[:, 4 * c:4 * c + 4, :])
            for c in range(2):
                nc.scalar.dma_start(wq_sb[:, 8 * c:8 * c + 8, :],
                                    wq_d[:, 8 * c:8 * c + 8, :])
                nc.scalar.dma_start(wk_sb[:, 8 * c:8 * c + 8, :],
                                    wk_d[:, 8 * c:8 * c + 8, :])
                nc.scalar.dma_start(wv_sb[:, 8 * c:8 * c + 8, :],
                                    wv_d[:, 8 * c:8 * c + 8, :])
            nc.scalar.dma_start(cos_sb[:], cos_d[:])
            nc.scalar.dma_start(sin_sb[:], sin_d[:])
            nc.scalar.dma_start(mask_sb[:], mask_d.rearrange("g p m -> p g m"))
            nc.scalar.dma_start(wo_sb[:], wo_d[:])

            # ---------- phase 1: QKV projection + RoPE ----------
            rope_deferred = []
            rope_chain = None
            with tc.tile_pool(name="ps1", bufs=1, space="PSUM") as ps1:
                for qc in range(NQC):
                    qsl = slice(qc * QC, (qc + 1) * QC)
                    qps = [ps1.tile([128, QC], F32, tag=f"qps{h}", name=f"qps{h}_{qc}") for h in range(HQL)]
                    kps = [ps1.tile([128, QC], F32, tag=f"kps{h}", name=f"kps{h}_{qc}") for h in range(HKL)]
                    vps = ps1.tile([128, 4, HKL * HD], F32, tag="vps")
                    for kt in range(NKT):
                        st, sp = (kt == 0), (kt == NKT - 1)
                        for h in range(HQL):
                            nc.tensor.matmul(qps[h][:], wq_sb[:, kt, h * HD:(h + 1) * HD],
                                             x_sb[:, kt, qsl], start=st, stop=sp)
                        for h in range(HKL):
                            nc.tensor.matmul(kps[h][:], wk_sb[:, kt, h * HD:(h + 1) * HD],
                                             x_sb[:, kt, qsl], start=st, stop=sp)
                        for sb in range(4):
                            # two 256-col outputs share one PSUM bank: only the
                            # bank's first writer may clear has_written (start)
                            nc.tensor.matmul(vps[:, sb, :],
                                             x_sb[:, kt, qc * QC + sb * 128:qc * QC + (sb + 1) * 128],
                                             wv_sb[:, kt, :],
                                             start=(st and sb % 2 == 0), stop=sp,
                                             skip_group_check=True)

                    # drain: one fast ACT copy per tile frees the PSUM bank;
                    # RoPE then runs SBUF-side in bf16 on the DVE fast modes.
                    # k0,q0 first: the first attention scores need them
                    evacs = []
                    for i, (ps, dst) in enumerate(
                            [(kps[0], kt_sb[0]), (qps[0], qt_sb[0]),
                             (kps[1], kt_sb[1]), (qps[1], qt_sb[1]),
                             (qps[2], qt_sb[2]), (qps[3], qt_sb[3])]):
                        qsb = rtmp.tile([128, QC], BF16, tag="evac",
                                        name=f"evac{qc}_{i}")
                        nc.scalar.copy(qsb[:], ps[:])
                        evacs.append((qsb, dst))
                    for sb in range(4):
                        nc.scalar.copy(v_sb[:, qc * 4 + sb, :], vps[:, sb, :])
                    def rope_chain(qsb, dst, qsl, tag):
                        qsw = rtmp.tile([128, QC], BF16, tag="swap",
                                        name=f"swap{tag}")
                        nc.vector.tensor_copy(qsw[0:64, :], qsb[64:128, :])
                        nc.vector.tensor_copy(qsw[64:128, :], qsb[0:64, :])
                        a_t = rtmp.tile([128, QC], BF16, tag="ropeA")
                        nc.vector.tensor_tensor(a_t[:], qsb[:], cos_sb[:, qsl], OP.mult)
                        b_t = rtmp.tile([128, QC], BF16, tag="ropeB")
                        nc.vector.tensor_tensor(b_t[:], qsw[:], sin_sb[:, qsl], OP.mult)
                        nc.vector.tensor_tensor(dst[:, qsl], a_t[:], b_t[:], OP.add)

                    for i, (qsb, dst) in enumerate(evacs):
                        rope_chain(qsb, dst, qsl, f"{qc}_{i}")

            # ---------- phase 2: attention (+ pipelined output proj) ----------
            def proj_unit(sb, dcp, ps_pool):
                # one (s-block, D-half) slice of the output projection;
                # deferred one q-chunk so all heads' O^T columns are ready
                fps = ps_pool.tile([128, 2, QC], F32, tag="sps",
                                   name=f"fps{sb}_{dcp}")
                for j in range(2):
                    dc = 2 * dcp + j
                    for h in range(HQL):
                        nc.tensor.matmul(
                            fps[:, j, :],
                            ot_sb[h][:, sb * 128:(sb + 1) * 128],
                            wo_sb[:, h, dc * QC:(dc + 1) * QC],
                            start=(h == 0), stop=(h == HQL - 1))
                o_sb = outp.tile([128, 2, QC], BF16, tag="osb")
                # evac on DVE only: ACT must stay free for the exp
                # stream (any ACT detour stalls PV groups downstream)
                nc.vector.tensor_copy(
                    o_sb[:].rearrange("p a b -> p (a b)"),
                    fps[:].rearrange("p a b -> p (a b)"))
                nc.sync.dma_start(
                    out_d[sb, :, dcp * 2 * QC:(dcp + 1) * 2 * QC],
                    o_sb[:].rearrange("p a b -> p (a b)"))

            with (
                tc.tile_pool(name="ps2s", bufs=2, space="PSUM") as ps2s,
                tc.tile_pool(name="ps2o", bufs=2, space="PSUM") as ps2o,
                tc.tile_pool(name="ps2z", bufs=2, space="PSUM") as ps2z,
            ):
                def emit_finalize(pend):
                    # 1/Z broadcast + O^T normalize for a PREVIOUS (qc,h):
                    # emitted one group into the next head so the PE's rb
                    # matmul never waits on the DVE reciprocal chain
                    p_ops, p_rz_r, p_h, p_qsl, p_qc, p_hh = pend
                    rb_ps = ps2z.tile([128, QC], F32, tag="zr",
                                      name=f"rbps{p_qc}_{p_hh}")
                    nc.tensor.matmul(rb_ps[:], ones_r[:], p_rz_r[:],
                                     start=True, stop=True)
                    rb_sb = small.tile([128, QC], F32, tag="rbsb")
                    nc.vector.tensor_copy(rb_sb[:], rb_ps[:])
                    nc.vector.tensor_tensor(p_h[:, p_qsl], p_ops[:], rb_sb[:],
                                            OP.mult)

                pending = None
                proj_q = []          # proj units of the previously done qc
                # qc descending: the 8-group heads of qc=3 keep the PE busy
                # right after the phase transition while DVE drains the
                # phase-1 RoPE tail and the first reciprocal chains
                for qc in [3, 2, 1, 0]:
                    for h in range(HQL):
                        kv = h // 2
                        qsl = slice(qc * QC, (qc + 1) * QC)
                        ops_t = ps2o.tile([128, QC], F32, tag="ops")
                        zps_t = ps2z.tile([128, QC], F32, tag="zr",
                                          name=f"zps{qc}_{h}")
                        ngrp = 2 * qc + 2          # groups of 2 k-blocks

                        def emit_pv(g, e_t):
                            # PV + Z for group g (runs one group behind the
                            # scores stream so the PE never waits on exp)
                            for j in range(2):
                                kb = 2 * g + j
                                off = (kb - 4 * qc) * 128 if kb >= 4 * qc else 0
                                st = (kb == 0)
                                sp = (kb == 4 * qc + 3)
                                nc.tensor.matmul(
                                    ops_t[:, off:], v_sb[:, kb, kv * HD:(kv + 1) * HD],
                                    e_t[:, j, off:], start=st, stop=sp,
                                    skip_group_check=True)
                                nc.tensor.matmul(
                                    zps_t[0:1, off:], ones_bf[:], e_t[:, j, off:],
                                    start=st, stop=sp, skip_group_check=True)

                        prev_pv = None
                        for g in range(ngrp):
                            diag = (g >= 2 * qc)   # last two groups touch diagonal
                            sps = ps2s.tile([128, 2, QC], F32, tag="sps")
                            e_t = epool.tile([128, 2, QC], BF16, tag="etile")
                            for j in range(2):
                                kb = 2 * g + j
                                # full q-width even on diagonal blocks: the
                                # mask zeroes invalid columns, and a fully
                                # written tile avoids uninit-PSUM reads in exp
                                nc.tensor.matmul(
                                    sps[:, j, :], kt_sb[kv][:, kb * 128:(kb + 1) * 128],
                                    qt_sb[h][:, qsl],
                                    start=True, stop=True)
                            nc.scalar.activation(
                                e_t[:].rearrange("p a b -> p (a b)"),
                                sps[:].rearrange("p a b -> p (a b)"),
                                AF.Exp, scale=SCALE)
                            if diag:
                                gi = g - 2 * qc
                                nc.vector.tensor_tensor(
                                    e_t[:].rearrange("p a b -> p (a b)"),
                                    e_t[:].rearrange("p a b -> p (a b)"),
                                    mask_sb[:, gi, :], OP.mult)
                            if prev_pv is not None:
                                emit_pv(*prev_pv)
                                if g == 1 and pending is not None:
                                    emit_finalize(pending)
                                    pending = None
                            prev_pv = (g, e_t)
                        emit_pv(*prev_pv)

                        # Z drain on DVE (reciprocal chain runs while the
                        # next head's first group streams on PE)
                        z_sb = small.tile([1, QC], F32, tag="zsb")
                        nc.vector.tensor_copy(z_sb[:], zps_t[0:1, :])
                        rz = small.tile([1, QC], F32, tag="rz")
                        nc.vector.reciprocal_approx_fast(rz[:], z_sb[:])
                        rz_r = small.tile([1, QC], F32R, tag="rzr")
                        nc.vector.tensor_copy(rz_r[:], rz[:])
                        pending = (ops_t, rz_r, ot_sb[h], qsl, qc, h)

                    # bulk-emit the PREVIOUS qc's projection (its last head
                    # was finalized during this qc's first head)
                    while proj_q:
                        proj_unit(*proj_q.pop(0), ps2s)
                    proj_q = [(sb, dcp) for sb in range(4 * qc, 4 * qc + 4)
                              for dcp in range(2)]
                emit_finalize(pending)
                while proj_q:
                    proj_unit(*proj_q.pop(0), ps2s)

    nc.compile()
    return nc


_NC_CACHE = None


def _get_nc():
    global _NC_CACHE
    if _NC_CACHE is None:
        _NC_CACHE = _build_nc()
    return _NC_CACHE


def _rope_tables():
    inv = 1.0 / (ROPE_THETA ** (np.arange(0, HD, 2, dtype=np.float64) / HD))  # [64]
    t = np.arange(S, dtype=np.float64)
    ang = np.outer(inv, t)                      # [64, S]
    cos = np.cos(ang).astype(np.float32)
    sin = np.sin(ang).astype(np.float32)
    cos128 = np.concatenate([cos, cos], axis=0).astype(BF)  # [128, S]
    sinM = np.concatenate([-sin, sin], axis=0).astype(BF)
    return cos128, sinM


def _masks():
    # dmask[g] covers a group of 2 k-blocks at diagonal offsets (2g*128,(2g+1)*128)
    q = np.arange(QC)
    m = np.zeros((2, 128, 2 * QC), np.float32)
    for g in range(2):
        for j in range(2):
            off = (2 * g + j) * 128
            k = np.arange(128) + off
            m[g, :, j * QC:(j + 1) * QC] = (k[:, None] <= q[None, :])
    return m.astype(BF)


def prepare_inputs(x, wq, wk, wv, wo):
    """Build the 8 per-core input dicts from full inputs."""
    perm = np.concatenate([np.arange(0, HD, 2), np.arange(1, HD, 2)])
    cos128, sinM = _rope_tables()
    dmask = _masks()

    x = np.asarray(x, np.float32)
    wq = np.asarray(wq, np.float32).reshape(HQ, HD, D)[:, perm, :]
    wk = np.asarray(wk, np.float32).reshape(HKV, HD, D)[:, perm, :]
    wv = np.asarray(wv, np.float32).reshape(HKV, HD, D)
    wo = np.asarray(wo, np.float32)              # [D, HQ*HD]

    in_maps = []
    for c in range(NCORES):
        b, hg = divmod(c, TP)
        qh = slice(hg * HQL, (hg + 1) * HQL)
        kh = slice(hg * HKL, (hg + 1) * HKL)
        # p-major: [128, NKT, *] so per-partition DRAM runs are 4KB+
        xT = np.ascontiguousarray(
            x[b].T.reshape(NKT, 128, S).transpose(1, 0, 2)).astype(BF)
        wq_t = np.ascontiguousarray(
            wq[qh].reshape(HQL * HD, D).T.reshape(NKT, 128, HQL * HD)
            .transpose(1, 0, 2)).astype(BF)
        wk_t = np.ascontiguousarray(
            wk[kh].reshape(HKL * HD, D).T.reshape(NKT, 128, HKL * HD)
            .transpose(1, 0, 2)).astype(BF)
        wv_t = np.ascontiguousarray(
            wv[kh].reshape(HKL * HD, D).T.reshape(NKT, 128, HKL * HD)
            .transpose(1, 0, 2)).astype(BF)
        wo_t = np.ascontiguousarray(
            wo[:, hg * HQL * HD:(hg + 1) * HQL * HD].T.reshape(HQL, HD, D)
            .transpose(1, 0, 2)).astype(BF)
        in_maps.append({
            "xT": xT, "wq_t": wq_t, "wk_t": wk_t, "wv_t": wv_t, "wo_t": wo_t,
            "cos128": cos128, "sinM": sinM, "dmask": dmask,
        })
    return in_maps


def _install_ntff_hook():
    """The agent image's antenv lacks axon_hooks; synthesize it so
    run_bass_kernel_spmd(trace=True) can capture NTFF profiles."""
    import sys as _sys
    import types, contextlib, ctypes

    if "antenv.axon_hooks" in _sys.modules:
        return
    so_path = "/opt/axon/libaxon_pjrt.so"
    lib = ctypes.CDLL(so_path)
    if not hasattr(lib, "axon_start_nrt_profile"):
        return
    lib.axon_start_nrt_profile.argtypes = [ctypes.POINTER(ctypes.c_int64),
                                           ctypes.c_size_t]
    lib.axon_start_nrt_profile.restype = ctypes.c_int64
    lib.axon_stop_nrt_profile.argtypes = [ctypes.c_char_p]
    lib.axon_stop_nrt_profile.restype = ctypes.c_int64

    @contextlib.contextmanager
    def _hook(output_dir, device_ids):
        import jax
        jax.devices()
        if device_ids:
            ids = (ctypes.c_int64 * len(device_ids))(*device_ids)
            rc = lib.axon_start_nrt_profile(ids, len(device_ids))
        else:
            rc = lib.axon_start_nrt_profile(None, 0)
        if rc != 0:
            raise RuntimeError(f"axon_start_nrt_profile rc={rc}")
        try:
            yield
        finally:
            n = lib.axon_stop_nrt_profile(str(output_dir).encode())
            print(f"ntff profile: {n} file(s) written to {output_dir}",
                  file=_sys.stderr)

    mod = types.ModuleType("antenv.axon_hooks")
    mod.get_axon_ntff_profile_hook = lambda: _hook
    mod.set_axon_ntff_profile_hook = lambda h: None
    _sys.modules["antenv.axon_hooks"] = mod
    try:
        import antenv
        antenv.axon_hooks = mod
    except ImportError:
        pass


def kernel(x, wq, wk, wv, wo, _trace=False, _trace_cores=None):
    in_maps = prepare_inputs(x, wq, wk, wv, wo)
    if _trace:
        _install_ntff_hook()
    nc = _get_nc()
    res = run_bass_kernel_spmd(
        nc, in_maps, core_ids=list(range(NCORES)),
        trace=_trace, trace_cores=_trace_cores)
    out = np.zeros((B, S, D), np.float32)
    for c in range(NCORES):
        b = c // TP
        out[b] += res.results[c]["out"].reshape(S, D).astype(np.float32)
    kernel.last_results = res
    return out


if __name__ == "__main__":
    rng = np.random.default_rng(0)
    x = rng.standard_normal((B, S, D), dtype=np.float32)
    sc = 1.0 / np.sqrt(D)
    wq = (rng.standard_normal((HQ * HD, D), dtype=np.float32) * sc)
    wk = (rng.standard_normal((HKV * HD, D), dtype=np.float32) * sc)
    wv = (rng.standard_normal((HKV * HD, D), dtype=np.float32) * sc)
    wo = (rng.standard_normal((D, HQ * HD), dtype=np.float32) * sc)
    out = kernel(x, wq, wk, wv, wo)
    print("ran", out.shape, out.dtype, float(np.abs(out).mean()))


# revision 36
# speedup vs baseline: 1.0088x; 1.0088x over previous
"""Causal self-attention (RoPE, GQA) on 8 Trainium2 NeuronCores.

Sharding: 2-way data-parallel over batch x 4-way tensor-parallel over heads.
Core c handles batch c//4 and head-group c%4 (4 q-heads, 2 kv-heads).
Each core computes its partial output projection (wo row-shard); the host
sums the 4 partials per batch (the "all-reduce" happens in the unshard step).

Layouts (all transposed so no on-device transposes are ever needed):
  - x is fed as xT [D, S] in bf16; Q/K are produced as [head_dim, S] via
    lhsT=weight-slice, rhs=xT; V is produced as [S, dv] via lhsT=xT-slice.
  - RoPE: weight rows are pre-permuted (even components -> rows 0:64,
    odd -> rows 64:128) so rotation is elementwise on row halves.
  - scores are computed transposed [k, q] so softmax numerator feeds the
    PV matmul directly; Z (denominator) via a ones-vector matmul; the
    normalization 1/Z via reciprocal on DVE plus a K=1 broadcast matmul.

PSUM plan (8 banks): scores 2-bank tiles double-buffered (4), O^T
accumulator double-buffered (2), a 2-bank ring shared by the Z accumulator
and the 1/Z broadcast (2). The output projection is deferred one q-chunk
so its inputs are always long-ready, and its PSUM evac alternates between
the ACT and DVE engines to keep ACT free for the exp stream.

Scheduling constraint honored throughout: a DVE TensorTensor can carry at
most ONE sync-wait, so every TT here has at most one freshly-produced
cross-engine operand.
"""

import sys
import numpy as np
import ml_dtypes

sys.path.insert(0, "/opt/trn_rl_repo")

import concourse.bass as bass
import concourse.bacc as bacc
import concourse.mybir as mybir
from concourse import tile
from concourse.bass_utils import run_bass_kernel_spmd

F32 = mybir.dt.float32
F32R = mybir.dt.float32r
BF16 = mybir.dt.bfloat16
AF = mybir.ActivationFunctionType
OP = mybir.AluOpType

B, S, D = 2, 2048, 2048
HQ, HKV, HD = 16, 8, 128
ROPE_THETA = 10000.0
NCORES, TP = 8, 4
HQL, HKL = HQ // TP, HKV // TP        # 4 q heads, 2 kv heads per core
NKT = D // 128                        # 16 contraction tiles
QC = 512                              # q-chunk width
NQC = S // QC                         # 4 q chunks
NSB = S // 128                        # 16 s-blocks
SCALE = 1.0 / float(np.sqrt(HD))
BF = ml_dtypes.bfloat16


def _build_nc():
    nc = bacc.Bacc("TRN2", target_bir_lowering=False)

    # p-major layouts: every DMA moves >=4KB contiguous per partition row
    # (1KB rows are descriptor-rate-bound at ~100ns/packet on the HWDGE)
    xT_d = nc.dram_tensor("xT", [128, NKT, S], BF16, kind="ExternalInput")
    wq_d = nc.dram_tensor("wq_t", [128, NKT, HQL * HD], BF16, kind="ExternalInput")
    wk_d = nc.dram_tensor("wk_t", [128, NKT, HKL * HD], BF16, kind="ExternalInput")
    wv_d = nc.dram_tensor("wv_t", [128, NKT, HKL * HD], BF16, kind="ExternalInput")
    wo_d = nc.dram_tensor("wo_t", [128, HQL, D], BF16, kind="ExternalInput")
    cos_d = nc.dram_tensor("cos128", [128, S], BF16, kind="ExternalInput")
    sin_d = nc.dram_tensor("sinM", [128, S], BF16, kind="ExternalInput")
    mask_d = nc.dram_tensor("dmask", [2, 128, 2 * QC], BF16, kind="ExternalInput")
    out_d = nc.dram_tensor("out", [NSB, 128, D], BF16, kind="ExternalOutput")

    with tile.TileContext(nc) as tc:
        with (
            tc.tile_pool(name="resident", bufs=1) as res,
            tc.tile_pool(name="ropetmp", bufs=2) as rtmp,
            tc.tile_pool(name="epool", bufs=3) as epool,
            tc.tile_pool(name="small", bufs=2) as small,
            tc.tile_pool(name="outp", bufs=2) as outp,
        ):
            # ---------- resident tiles: x and weights live in SBUF whole;
            # DMAs are chunked so the first matmuls start after ~2us ----------
            x_sb = res.tile([128, NKT, S], BF16)
            wq_sb = res.tile([128, NKT, HQL * HD], BF16)
            wk_sb = res.tile([128, NKT, HKL * HD], BF16)
            wv_sb = res.tile([128, NKT, HKL * HD], BF16)
            wo_sb = res.tile([128, HQL, D], BF16)
            cos_sb = res.tile([128, S], BF16)
            sin_sb = res.tile([128, S], BF16)
            mask_sb = res.tile([128, 2, 2 * QC], BF16)

            ones_bf = res.tile([128, 1], BF16)
            nc.vector.memset(ones_bf[:], 1.0)
            ones_rf = res.tile([1, 128], F32)
            nc.vector.memset(ones_rf[:], 1.0)
            ones_r = res.tile([1, 128], F32R)
            nc.vector.tensor_copy(ones_r[:], ones_rf[:])

            # outputs of phase 1 (resident through phase 2/3)
            qt_sb = [res.tile([128, S], BF16, tag=f"qt{h}", name=f"qt{h}") for h in range(HQL)]
            kt_sb = [res.tile([128, S], BF16, tag=f"kt{h}", name=f"kt{h}") for h in range(HKL)]
            v_sb = res.tile([128, NSB, HKL * HD], BF16)
            ot_sb = [res.tile([128, S], BF16, tag=f"ot{h}", name=f"ot{h}") for h in range(HQL)]

            # ---------- input DMAs: x per k-tile on the sync queue (4KB
            # rows), weights in 4-ktile chunks on the scalar queue ----------
            # first 4 k-tiles individually (earliest availability), the rest
            # in 4-ktile chunks: 16KB contiguous per partition row keeps the
            # HWDGE descriptor count low (the real DMA-rate limiter)
            for kt in range(4):
                nc.sync.dma_start(x_sb[:, kt, :], xT_d[:, kt, :])
            for c in range(1, 4):
                nc.sync.dma_start(x_sb[:, 4 * c:4 * c + 4, :],
                                  xT_d[:, 4 * c:4 * c + 4, :])
            for c in range(2):
                nc.scalar.dma_start(wq_sb[:, 8 * c:8 * c + 8, :],
                                    wq_d[:, 8 * c:8 * c + 8, :])
                nc.scalar.dma_start(wk_sb[:, 8 * c:8 * c + 8, :],
                                    wk_d[:, 8 * c:8 * c + 8, :])
                nc.scalar.dma_start(wv_sb[:, 8 * c:8 * c + 8, :],
                                    wv_d[:, 8 * c:8 * c + 8, :])
            nc.scalar.dma_start(cos_sb[:], cos_d[:])
            nc.scalar.dma_start(sin_sb[:], sin_d[:])
            nc.scalar.dma_start(mask_sb[:], mask_d.rearrange("g p m -> p g m"))
            nc.scalar.dma_start(wo_sb[:], wo_d[:])

            # ---------- phase 1: QKV projection + RoPE ----------
            rope_deferred = []
            rope_chain = None
            with tc.tile_pool(name="ps1", bufs=1, space="PSUM") as ps1:
                for qc in range(NQC):
                    qsl = slice(qc * QC, (qc + 1) * QC)
                    qps = [ps1.tile([128, QC], F32, tag=f"qps{h}", name=f"qps{h}_{qc}") for h in range(HQL)]
                    kps = [ps1.tile([128, QC], F32, tag=f"kps{h}", name=f"kps{h}_{qc}") for h in range(HKL)]
                    vps = ps1.tile([128, 4, HKL * HD], F32, tag="vps")
                    for kt in range(NKT):
                        st, sp = (kt == 0), (kt == NKT - 1)
                        for h in range(HQL):
                            nc.tensor.matmul(qps[h][:], wq_sb[:, kt, h * HD:(h + 1) * HD],
                                             x_sb[:, kt, qsl], start=st, stop=sp)
                        for h in range(HKL):
                            nc.tensor.matmul(kps[h][:], wk_sb[:, kt, h * HD:(h + 1) * HD],
                                             x_sb[:, kt, qsl], start=st, stop=sp)
                        for sb in range(4):
                            # two 256-col outputs share one PSUM bank: only the
                            # bank's first writer may clear has_written (start)
                            nc.tensor.matmul(vps[:, sb, :],
                                             x_sb[:, kt, qc * QC + sb * 128:qc * QC + (sb + 1) * 128],
                                             wv_sb[:, kt, :],
                                             start=(st and sb % 2 == 0), stop=sp,
                                             skip_group_check=True)

                    # drain: one fast ACT copy per tile frees the PSUM bank;
                    # RoPE then runs SBUF-side in bf16 on the DVE fast modes.
                    # k0,q0 first: the first attention scores need them
                    evacs = []
                    for i, (ps, dst) in enumerate(
                            [(kps[0], kt_sb[0]), (qps[0], qt_sb[0]),
                             (kps[1], kt_sb[1]), (qps[1], qt_sb[1]),
                             (qps[2], qt_sb[2]), (qps[3], qt_sb[3])]):
                        qsb = rtmp.tile([128, QC], BF16, tag="evac",
                                        name=f"evac{qc}_{i}")
                        if i >= 4 and qc < NQC - 1:
                            # q2/q3 evacs on DVE: the 10-bank ACT drain
                            # (~4.3us serial) can't free banks before the
                            # next qc's matmuls need them
                            nc.vector.tensor_copy(qsb[:], ps[:])
                        else:
                            nc.scalar.copy(qsb[:], ps[:])
                        evacs.append((qsb, dst))
                    for sb in range(4):
                        nc.scalar.copy(v_sb[:, qc * 4 + sb, :], vps[:, sb, :])
                    def rope_chain(qsb, dst, qsl, tag):
                        qsw = rtmp.tile([128, QC], BF16, tag="swap",
                                        name=f"swap{tag}")
                        nc.vector.tensor_copy(qsw[0:64, :], qsb[64:128, :])
                        nc.vector.tensor_copy(qsw[64:128, :], qsb[0:64, :])
                        a_t = rtmp.tile([128, QC], BF16, tag="ropeA")
                        nc.vector.tensor_tensor(a_t[:], qsb[:], cos_sb[:, qsl], OP.mult)
                        b_t = rtmp.tile([128, QC], BF16, tag="ropeB")
                        nc.vector.tensor_tensor(b_t[:], qsw[:], sin_sb[:, qsl], OP.mult)
                        nc.vector.tensor_tensor(dst[:, qsl], a_t[:], b_t[:], OP.add)

                    for i, (qsb, dst) in enumerate(evacs):
                        if qc == NQC - 1 and i >= 4:
                            # defer ONLY the last two ring allocations (q2,q3)
                            # -- no later "evac" alloc waits on them, so the
                            # phase boundary stays unserialized; their heads
                            # run 2nd/3rd in phase 2's descending-qc order
                            rope_deferred.append((qsb, dst, qsl, f"{qc}_{i}"))
                        else:
                            rope_chain(qsb, dst, qsl, f"{qc}_{i}")

            # ---------- phase 2: attention (+ pipelined output proj) ----------
            def proj_unit(sb, dcp, ps_pool):
                # one (s-block, D-half) slice of the output projection;
                # deferred one q-chunk so all heads' O^T columns are ready
                fps = ps_pool.tile([128, 2, QC], F32, tag="sps",
                                   name=f"fps{sb}_{dcp}")
                for j in range(2):
                    dc = 2 * dcp + j
                    for h in range(HQL):
                        nc.tensor.matmul(
                            fps[:, j, :],
                            ot_sb[h][:, sb * 128:(sb + 1) * 128],
                            wo_sb[:, h, dc * QC:(dc + 1) * QC],
                            start=(h == 0), stop=(h == HQL - 1))
                o_sb = outp.tile([128, 2, QC], BF16, tag="osb")
                # evac on DVE only: ACT must stay free for the exp
                # stream (any ACT detour stalls PV groups downstream)
                nc.vector.tensor_copy(
                    o_sb[:].rearrange("p a b -> p (a b)"),
                    fps[:].rearrange("p a b -> p (a b)"))
                # alternate HWDGE queues so the final output flush
                # drains in parallel
                q = nc.sync if dcp == 0 else nc.scalar
                q.dma_start(
                    out_d[sb, :, dcp * 2 * QC:(dcp + 1) * 2 * QC],
                    o_sb[:].rearrange("p a b -> p (a b)"))

            with (
                tc.tile_pool(name="ps2s", bufs=2, space="PSUM") as ps2s,
                tc.tile_pool(name="ps2o", bufs=2, space="PSUM") as ps2o,
                tc.tile_pool(name="ps2z", bufs=2, space="PSUM") as ps2z,
            ):
                def emit_finalize(pend):
                    # 1/Z broadcast + O^T normalize for a PREVIOUS (qc,h):
                    # emitted one group into the next head so the PE's rb
                    # matmul never waits on the DVE reciprocal chain
                    p_ops, p_rz_r, p_h, p_qsl, p_qc, p_hh = pend
                    rb_ps = ps2z.tile([128, QC], F32, tag="zr",
                                      name=f"rbps{p_qc}_{p_hh}")
                    nc.tensor.matmul(rb_ps[:], ones_r[:], p_rz_r[:],
                                     start=True, stop=True)
                    rb_sb = small.tile([128, QC], F32, tag="rbsb")
                    nc.vector.tensor_copy(rb_sb[:], rb_ps[:])
                    nc.vector.tensor_tensor(p_h[:, p_qsl], p_ops[:], rb_sb[:],
                                            OP.mult)

                pending = None
                proj_q = []          # proj units of the previously done qc
                # qc descending: the 8-group heads of qc=3 keep the PE busy
                # right after the phase transition while DVE drains the
                # phase-1 RoPE tail and the first reciprocal chains
                for qc in [3, 2, 1, 0]:
                    for h in range(HQL):
                        kv = h // 2
                        qsl = slice(qc * QC, (qc + 1) * QC)
                        ops_t = ps2o.tile([128, QC], F32, tag="ops")
                        zps_t = ps2z.tile([128, QC], F32, tag="zr",
                                          name=f"zps{qc}_{h}")
                        ngrp = 2 * qc + 2          # groups of 2 k-blocks

                        def emit_pv(g, e_t):
                            # PV + Z for group g (runs one group behind the
                            # scores stream so the PE never waits on exp)
                            for j in range(2):
                                kb = 2 * g + j
                                off = (kb - 4 * qc) * 128 if kb >= 4 * qc else 0
                                st = (kb == 0)
                                sp = (kb == 4 * qc + 3)
                                nc.tensor.matmul(
                                    ops_t[:, off:], v_sb[:, kb, kv * HD:(kv + 1) * HD],
                                    e_t[:, j, off:], start=st, stop=sp,
                                    skip_group_check=True)
                                nc.tensor.matmul(
                                    zps_t[0:1, off:], ones_bf[:], e_t[:, j, off:],
                                    start=st, stop=sp, skip_group_check=True)

                        prev_pv = None
                        for g in range(ngrp):
                            diag = (g >= 2 * qc)   # last two groups touch diagonal
                            sps = ps2s.tile([128, 2, QC], F32, tag="sps")
                            e_t = epool.tile([128, 2, QC], BF16, tag="etile")
                            for j in range(2):
                                kb = 2 * g + j
                                # full q-width even on diagonal blocks: the
                                # mask zeroes invalid columns, and a fully
                                # written tile avoids uninit-PSUM reads in exp
                                nc.tensor.matmul(
                                    sps[:, j, :], kt_sb[kv][:, kb * 128:(kb + 1) * 128],
                                    qt_sb[h][:, qsl],
                                    start=True, stop=True)
                            nc.scalar.activation(
                                e_t[:].rearrange("p a b -> p (a b)"),
                                sps[:].rearrange("p a b -> p (a b)"),
                                AF.Exp, scale=SCALE)
                            if diag:
                                gi = g - 2 * qc
                                nc.vector.tensor_tensor(
                                    e_t[:].rearrange("p a b -> p (a b)"),
                                    e_t[:].rearrange("p a b -> p (a b)"),
                                    mask_sb[:, gi, :], OP.mult)
                            if prev_pv is not None:
                                emit_pv(*prev_pv)
                                if g == 1 and pending is not None:
                                    emit_finalize(pending)
                                    pending = None
                            prev_pv = (g, e_t)
                        emit_pv(*prev_pv)

                        # Z drain on DVE (reciprocal chain runs while the
                        # next head's first group streams on PE)
                        z_sb = small.tile([1, QC], F32, tag="zsb")
                        nc.vector.tensor_copy(z_sb[:], zps_t[0:1, :])
                        rz = small.tile([1, QC], F32, tag="rz")
                        nc.vector.reciprocal_approx_fast(rz[:], z_sb[:])
                        rz_r = small.tile([1, QC], F32R, tag="rzr")
                        nc.vector.tensor_copy(rz_r[:], rz[:])
                        pending = (ops_t, rz_r, ot_sb[h], qsl, qc, h)

                        if rope_deferred:
                            rope_chain(*rope_deferred.pop(0))

                    # bulk-emit the PREVIOUS qc's projection (its last head
                    # was finalized during this qc's first head)
                    while proj_q:
                        proj_unit(*proj_q.pop(0), ps2s)
                    proj_q = [(sb, dcp) for sb in range(4 * qc, 4 * qc + 4)
                              for dcp in range(2)]
                emit_finalize(pending)
                while proj_q:
                    proj_unit(*proj_q.pop(0), ps2s)

    nc.compile()
    return nc


_NC_CACHE = None


def _get_nc():
    global _NC_CACHE
    if _NC_CACHE is None:
        _NC_CACHE = _build_nc()
    return _NC_CACHE


def _rope_tables():
    inv = 1.0 / (ROPE_THETA ** (np.arange(0, HD, 2, dtype=np.float64) / HD))  # [64]
    t = np.arange(S, dtype=np.float64)
    ang = np.outer(inv, t)                      # [64, S]
    cos = np.cos(ang).astype(np.float32)
    sin = np.sin(ang).astype(np.float32)
    cos128 = np.concatenate([cos, cos], axis=0).astype(BF)  # [128, S]
    sinM = np.concatenate([-sin, sin], axis=0).astype(BF)
    return cos128, sinM


def _masks():
    # dmask[g] covers a group of 2 k-blocks at diagonal offsets (2g*128,(2g+1)*128)
    q = np.arange(QC)
    m = np.zeros((2, 128, 2 * QC), np.float32)
    for g in range(2):
        for j in range(2):
            off = (2 * g + j) * 128
            k = np.arange(128) + off
            m[g, :, j * QC:(j + 1) * QC] = (k[:, None] <= q[None, :])
    return m.astype(BF)


def prepare_inputs(x, wq, wk, wv, wo):
    """Build the 8 per-core input dicts from full inputs."""
    perm = np.concatenate([np.arange(0, HD, 2), np.arange(1, HD, 2)])
    cos128, sinM = _rope_tables()
    dmask = _masks()

    x = np.asarray(x, np.float32)
    wq = np.asarray(wq, np.float32).reshape(HQ, HD, D)[:, perm, :]
    wk = np.asarray(wk, np.float32).reshape(HKV, HD, D)[:, perm, :]
    wv = np.asarray(wv, np.float32).reshape(HKV, HD, D)
    wo = np.asarray(wo, np.float32)              # [D, HQ*HD]

    in_maps = []
    for c in range(NCORES):
        b, hg = divmod(c, TP)
        qh = slice(hg * HQL, (hg + 1) * HQL)
        kh = slice(hg * HKL, (hg + 1) * HKL)
        # p-major: [128, NKT, *] so per-partition DRAM runs are 4KB+
        xT = np.ascontiguousarray(
            x[b].T.reshape(NKT, 128, S).transpose(1, 0, 2)).astype(BF)
        wq_t = np.ascontiguousarray(
            wq[qh].reshape(HQL * HD, D).T.reshape(NKT, 128, HQL * HD)
            .transpose(1, 0, 2)).astype(BF)
        wk_t = np.ascontiguousarray(
            wk[kh].reshape(HKL * HD, D).T.reshape(NKT, 128, HKL * HD)
            .transpose(1, 0, 2)).astype(BF)
        wv_t = np.ascontiguousarray(
            wv[kh].reshape(HKL * HD, D).T.reshape(NKT, 128, HKL * HD)
            .transpose(1, 0, 2)).astype(BF)
        wo_t = np.ascontiguousarray(
            wo[:, hg * HQL * HD:(hg + 1) * HQL * HD].T.reshape(HQL, HD, D)
            .transpose(1, 0, 2)).astype(BF)
        in_maps.append({
            "xT": xT, "wq_t": wq_t, "wk_t": wk_t, "wv_t": wv_t, "wo_t": wo_t,
            "cos128": cos128, "sinM": sinM, "dmask": dmask,
        })
    return in_maps


def _install_ntff_hook():
    """The agent image's antenv lacks axon_hooks; synthesize it so
    run_bass_kernel_spmd(trace=True) can capture NTFF profiles."""
    import sys as _sys
    import types, contextlib, ctypes

    if "antenv.axon_hooks" in _sys.modules:
        return
    so_path = "/opt/axon/libaxon_pjrt.so"
    lib = ctypes.CDLL(so_path)
    if not hasattr(lib, "axon_start_nrt_profile"):
        return
    lib.axon_start_nrt_profile.argtypes = [ctypes.POINTER(ctypes.c_int64),
                                           ctypes.c_size_t]
    lib.axon_start_nrt_profile.restype = ctypes.c_int64
    lib.axon_stop_nrt_profile.argtypes = [ctypes.c_char_p]
    lib.axon_stop_nrt_profile.restype = ctypes.c_int64

    @contextlib.contextmanager
    def _hook(output_dir, device_ids):
        import jax
        jax.devices()
        if device_ids:
            ids = (ctypes.c_int64 * len(device_ids))(*device_ids)
            rc = lib.axon_start_nrt_profile(ids, len(device_ids))
        else:
            rc = lib.axon_start_nrt_profile(None, 0)
        if rc != 0:
            raise RuntimeError(f"axon_start_nrt_profile rc={rc}")
        try:
            yield
        finally:
            n = lib.axon_stop_nrt_profile(str(output_dir).encode())
            print(f"ntff profile: {n} file(s) written to {output_dir}",
                  file=_sys.stderr)

    mod = types.ModuleType("antenv.axon_hooks")
    mod.get_axon_ntff_profile_hook = lambda: _hook
    mod.set_axon_ntff_profile_hook = lambda h: None
    _sys.modules["antenv.axon_hooks"] = mod
    try:
        import antenv
        antenv.axon_hooks = mod
    except ImportError:
        pass


def kernel(x, wq, wk, wv, wo, _trace=False, _trace_cores=None):
    in_maps = prepare_inputs(x, wq, wk, wv, wo)
    if _trace:
        _install_ntff_hook()
    nc = _get_nc()
    res = run_bass_kernel_spmd(
        nc, in_maps, core_ids=list(range(NCORES)),
        trace=_trace, trace_cores=_trace_cores)
    out = np.zeros((B, S, D), np.float32)
    for c in range(NCORES):
        b = c // TP
        out[b] += res.results[c]["out"].reshape(S, D).astype(np.float32)
    kernel.last_results = res
    return out


if __name__ == "__main__":
    rng = np.random.default_rng(0)
    x = rng.standard_normal((B, S, D), dtype=np.float32)
    sc = 1.0 / np.sqrt(D)
    wq = (rng.standard_normal((HQ * HD, D), dtype=np.float32) * sc)
    wk = (rng.standard_normal((HKV * HD, D), dtype=np.float32) * sc)
    wv = (rng.standard_normal((HKV * HD, D), dtype=np.float32) * sc)
    wo = (rng.standard_normal((D, HQ * HD), dtype=np.float32) * sc)
    out = kernel(x, wq, wk, wv, wo)
    print("ran", out.shape, out.dtype, float(np.abs(out).mean()))


# revision 42
# speedup vs baseline: 1.1927x; 1.1823x over previous
"""Causal self-attention (RoPE, GQA) on 8 Trainium2 NeuronCores.

Sharding: 2-way data-parallel over batch x 4-way tensor-parallel over heads.
Core c handles batch c//4 and head-group c%4 (4 q-heads, 2 kv-heads).
Each core computes its partial output projection (wo row-shard); the host
sums the 4 partials per batch (the "all-reduce" happens in the unshard step).

Layouts (all transposed so no on-device transposes are ever needed):
  - x is fed as xT [D, S] in bf16; Q/K are produced as [head_dim, S] via
    lhsT=weight-slice, rhs=xT; V is produced as [S, dv] via lhsT=xT-slice.
  - RoPE: weight rows are pre-permuted (even components -> rows 0:64,
    odd -> rows 64:128) so rotation is elementwise on row halves.
  - scores are computed transposed [k, q] so softmax numerator feeds the
    PV matmul directly; Z (denominator) via a ones-vector matmul; the
    normalization 1/Z via reciprocal on DVE plus a K=1 broadcast matmul.

PSUM plan (8 banks): scores 2-bank tiles double-buffered (4), O^T
accumulator double-buffered (2), a 2-bank ring shared by the Z accumulator
and the 1/Z broadcast (2). The output projection is deferred one q-chunk
so its inputs are always long-ready, and its PSUM evac alternates between
the ACT and DVE engines to keep ACT free for the exp stream.

Scheduling constraint honored throughout: a DVE TensorTensor can carry at
most ONE sync-wait, so every TT here has at most one freshly-produced
cross-engine operand.
"""

import sys
import numpy as np
import ml_dtypes

sys.path.insert(0, "/opt/trn_rl_repo")

import concourse.bass as bass
import concourse.bacc as bacc
import concourse.mybir as mybir
from concourse import tile
from concourse.bass_utils import run_bass_kernel_spmd

F32 = mybir.dt.float32
F32R = mybir.dt.float32r
BF16 = mybir.dt.bfloat16
AF = mybir.ActivationFunctionType
OP = mybir.AluOpType

B, S, D = 2, 2048, 2048
HQ, HKV, HD = 16, 8, 128
ROPE_THETA = 10000.0
NCORES, TP = 8, 4
HQL, HKL = HQ // TP, HKV // TP        # 4 q heads, 2 kv heads per core
NKT = D // 128                        # 16 contraction tiles
QC = 512                              # q-chunk width
NQC = S // QC                         # 4 q chunks
NSB = S // 128                        # 16 s-blocks
SCALE = 1.0 / float(np.sqrt(HD))
BF = ml_dtypes.bfloat16


def _build_nc():
    nc = bacc.Bacc("TRN2", target_bir_lowering=False)

    # p-major layouts: every DMA moves >=4KB contiguous per partition row
    # (1KB rows are descriptor-rate-bound at ~100ns/packet on the HWDGE)
    xT_d = nc.dram_tensor("xT", [128, NKT, S], BF16, kind="ExternalInput")
    wq_d = nc.dram_tensor("wq_t", [128, NKT, HQL * HD], BF16, kind="ExternalInput")
    wk_d = nc.dram_tensor("wk_t", [128, NKT, HKL * HD], BF16, kind="ExternalInput")
    wv_d = nc.dram_tensor("wv_t", [128, NKT, HKL * HD], BF16, kind="ExternalInput")
    wo_d = nc.dram_tensor("wo_t", [128, HQL, D], BF16, kind="ExternalInput")
    cos_d = nc.dram_tensor("cos128", [128, S], BF16, kind="ExternalInput")
    sin_d = nc.dram_tensor("sinM", [128, S], BF16, kind="ExternalInput")
    mask_d = nc.dram_tensor("dmask", [2, 128, 2 * QC], BF16, kind="ExternalInput")
    out_d = nc.dram_tensor("out", [NSB, 128, D], BF16, kind="ExternalOutput")

    with tile.TileContext(nc) as tc:
        with (
            tc.tile_pool(name="resident", bufs=1) as res,
            tc.tile_pool(name="ropetmp", bufs=2) as rtmp,
            tc.tile_pool(name="epool", bufs=3) as epool,
            tc.tile_pool(name="small", bufs=2) as small,
            tc.tile_pool(name="outp", bufs=2) as outp,
        ):
            # ---------- resident tiles: x and weights live in SBUF whole;
            # DMAs are chunked so the first matmuls start after ~2us ----------
            x_sb = res.tile([128, NKT, S], BF16)
            wq_sb = res.tile([128, NKT, HQL * HD], BF16)
            wk_sb = res.tile([128, NKT, HKL * HD], BF16)
            wv_sb = res.tile([128, NKT, HKL * HD], BF16)
            wo_sb = res.tile([128, HQL, D], BF16)
            cos_sb = res.tile([128, S], BF16)
            sin_sb = res.tile([128, S], BF16)
            mask_sb = res.tile([128, 2, 2 * QC], BF16)

            ones_bf = res.tile([128, 1], BF16)
            nc.vector.memset(ones_bf[:], 1.0)
            ones_rf = res.tile([1, 128], F32)
            nc.vector.memset(ones_rf[:], 1.0)
            ones_r = res.tile([1, 128], F32R)
            nc.vector.tensor_copy(ones_r[:], ones_rf[:])

            # outputs of phase 1 (resident through phase 2/3)
            qt_sb = [res.tile([128, S], BF16, tag=f"qt{h}", name=f"qt{h}") for h in range(HQL)]
            kt_sb = [res.tile([128, S], BF16, tag=f"kt{h}", name=f"kt{h}") for h in range(HKL)]
            v_sb = res.tile([128, NSB, HKL * HD], BF16)
            ot_sb = [res.tile([128, S], BF16, tag=f"ot{h}", name=f"ot{h}") for h in range(HQL)]

            # ---------- input DMAs: x per k-tile on the sync queue (4KB
            # rows), weights in 4-ktile chunks on the scalar queue ----------
            # first 4 k-tiles individually (earliest availability), the rest
            # in 4-ktile chunks: 16KB contiguous per partition row keeps the
            # HWDGE descriptor count low (the real DMA-rate limiter)
            for kt in range(4):
                nc.sync.dma_start(x_sb[:, kt, :], xT_d[:, kt, :])
            for c in range(1, 4):
                nc.sync.dma_start(x_sb[:, 4 * c:4 * c + 4, :],
                                  xT_d[:, 4 * c:4 * c + 4, :])
            for c in range(2):
                nc.scalar.dma_start(wq_sb[:, 8 * c:8 * c + 8, :],
                                    wq_d[:, 8 * c:8 * c + 8, :])
                nc.scalar.dma_start(wk_sb[:, 8 * c:8 * c + 8, :],
                                    wk_d[:, 8 * c:8 * c + 8, :])
                nc.scalar.dma_start(wv_sb[:, 8 * c:8 * c + 8, :],
                                    wv_d[:, 8 * c:8 * c + 8, :])
            nc.scalar.dma_start(cos_sb[:], cos_d[:])
            nc.scalar.dma_start(sin_sb[:], sin_d[:])
            nc.scalar.dma_start(mask_sb[:], mask_d.rearrange("g p m -> p g m"))
            nc.scalar.dma_start(wo_sb[:], wo_d[:])

            # ---------- phase 1: QKV projection + RoPE ----------
            # qc order [0,1,3,2]: qc=3's RoPE chains (phase 2's first
            # inputs, descending order) clear the DVE during qc=2's ~35us
            # of matmuls instead of backing up at the phase boundary
            rope_deferred = []
            rope_chain = None
            with tc.tile_pool(name="ps1", bufs=1, space="PSUM") as ps1:
                for qc in [0, 1, 3, 2]:
                    qsl = slice(qc * QC, (qc + 1) * QC)
                    qps = [ps1.tile([128, QC], F32, tag=f"qps{h}", name=f"qps{h}_{qc}") for h in range(HQL)]
                    kps = [ps1.tile([128, QC], F32, tag=f"kps{h}", name=f"kps{h}_{qc}") for h in range(HKL)]
                    vps = ps1.tile([128, 4, HKL * HD], F32, tag="vps")
                    for kt in range(NKT):
                        st, sp = (kt == 0), (kt == NKT - 1)
                        for h in range(HQL):
                            nc.tensor.matmul(qps[h][:], wq_sb[:, kt, h * HD:(h + 1) * HD],
                                             x_sb[:, kt, qsl], start=st, stop=sp)
                        for h in range(HKL):
                            nc.tensor.matmul(kps[h][:], wk_sb[:, kt, h * HD:(h + 1) * HD],
                                             x_sb[:, kt, qsl], start=st, stop=sp)
                        for sb in range(4):
                            # two 256-col outputs share one PSUM bank: only the
                            # bank's first writer may clear has_written (start)
                            nc.tensor.matmul(vps[:, sb, :],
                                             x_sb[:, kt, qc * QC + sb * 128:qc * QC + (sb + 1) * 128],
                                             wv_sb[:, kt, :],
                                             start=(st and sb % 2 == 0), stop=sp,
                                             skip_group_check=True)

                    # drain: one fast ACT copy per tile frees the PSUM bank;
                    # RoPE then runs SBUF-side in bf16 on the DVE fast modes.
                    # k0,q0 first: the first attention scores need them
                    evacs = []
                    for i, (ps, dst) in enumerate(
                            [(kps[0], kt_sb[0]), (qps[0], qt_sb[0]),
                             (kps[1], kt_sb[1]), (qps[1], qt_sb[1]),
                             (qps[2], qt_sb[2]), (qps[3], qt_sb[3])]):
                        qsb = rtmp.tile([128, QC], BF16, tag="evac",
                                        name=f"evac{qc}_{i}")
                        if i >= 4 and qc != 3:
                            # q2/q3 evacs on DVE: the 10-bank ACT drain
                            # (~4.3us serial) can't free banks before the
                            # next qc's matmuls need them
                            nc.vector.tensor_copy(qsb[:], ps[:])
                        else:
                            nc.scalar.copy(qsb[:], ps[:])
                        evacs.append((qsb, dst))
                    for sb in range(4):
                        nc.scalar.copy(v_sb[:, qc * 4 + sb, :], vps[:, sb, :])
                    def rope_chain(qsb, dst, qsl, tag):
                        qsw = rtmp.tile([128, QC], BF16, tag="swap",
                                        name=f"swap{tag}")
                        nc.vector.tensor_copy(qsw[0:64, :], qsb[64:128, :])
                        nc.vector.tensor_copy(qsw[64:128, :], qsb[0:64, :])
                        a_t = rtmp.tile([128, QC], BF16, tag="ropeA")
                        nc.vector.tensor_tensor(a_t[:], qsb[:], cos_sb[:, qsl], OP.mult)
                        b_t = rtmp.tile([128, QC], BF16, tag="ropeB")
                        nc.vector.tensor_tensor(b_t[:], qsw[:], sin_sb[:, qsl], OP.mult)
                        nc.vector.tensor_tensor(dst[:, qsl], a_t[:], b_t[:], OP.add)

                    for i, (qsb, dst) in enumerate(evacs):
                        if qc == 2 and (i % 2 == 1 or i >= 4):
                            # qc=2 is processed LAST: its evac tiles are the
                            # final "evac"-ring allocations, so deferring its
                            # Q chains into phase 2 is ring-safe; they are
                            # needed only when attention reaches qc=2 (~40us
                            # in). K chains must stay: attention(qc=3) reads
                            # ALL kt columns immediately.
                            rope_deferred.append((qsb, dst, qsl, f"{qc}_{i}"))
                        else:
                            rope_chain(qsb, dst, qsl, f"{qc}_{i}")

            # ---------- phase 2: attention (+ pipelined output proj) ----------
            def proj_unit(sb, dcp, ps_pool):
                # one (s-block, D-half) slice of the output projection;
                # deferred one q-chunk so all heads' O^T columns are ready
                fps = ps_pool.tile([128, 2, QC], F32, tag="sps",
                                   name=f"fps{sb}_{dcp}")
                for j in range(2):
                    dc = 2 * dcp + j
                    for h in range(HQL):
                        nc.tensor.matmul(
                            fps[:, j, :],
                            ot_sb[h][:, sb * 128:(sb + 1) * 128],
                            wo_sb[:, h, dc * QC:(dc + 1) * QC],
                            start=(h == 0), stop=(h == HQL - 1))
                o_sb = outp.tile([128, 2, QC], BF16, tag="osb")
                # evac on DVE only: ACT must stay free for the exp
                # stream (any ACT detour stalls PV groups downstream)
                nc.vector.tensor_copy(
                    o_sb[:].rearrange("p a b -> p (a b)"),
                    fps[:].rearrange("p a b -> p (a b)"))
                # alternate HWDGE queues so the final output flush
                # drains in parallel
                q = nc.sync if dcp == 0 else nc.scalar
                q.dma_start(
                    out_d[sb, :, dcp * 2 * QC:(dcp + 1) * 2 * QC],
                    o_sb[:].rearrange("p a b -> p (a b)"))

            with (
                tc.tile_pool(name="ps2s", bufs=2, space="PSUM") as ps2s,
                tc.tile_pool(name="ps2o", bufs=2, space="PSUM") as ps2o,
                tc.tile_pool(name="ps2z", bufs=2, space="PSUM") as ps2z,
            ):
                def emit_finalize(pend):
                    # 1/Z broadcast + O^T normalize for a PREVIOUS (qc,h):
                    # emitted one group into the next head so the PE's rb
                    # matmul never waits on the DVE reciprocal chain
                    p_ops, p_rz_r, p_h, p_qsl, p_qc, p_hh = pend
                    rb_ps = ps2z.tile([128, QC], F32, tag="zr",
                                      name=f"rbps{p_qc}_{p_hh}")
                    nc.tensor.matmul(rb_ps[:], ones_r[:], p_rz_r[:],
                                     start=True, stop=True)
                    rb_sb = small.tile([128, QC], F32, tag="rbsb")
                    nc.vector.tensor_copy(rb_sb[:], rb_ps[:])
                    nc.vector.tensor_tensor(p_h[:, p_qsl], p_ops[:], rb_sb[:],
                                            OP.mult)

                pending = None
                proj_q = []          # proj units of the previously done qc
                # qc descending: the 8-group heads of qc=3 keep the PE busy
                # right after the phase transition while DVE drains the
                # phase-1 RoPE tail and the first reciprocal chains
                for qc in [3, 2, 1, 0]:
                    for h in range(HQL):
                        kv = h // 2
                        qsl = slice(qc * QC, (qc + 1) * QC)
                        ops_t = ps2o.tile([128, QC], F32, tag="ops")
                        zps_t = ps2z.tile([128, QC], F32, tag="zr",
                                          name=f"zps{qc}_{h}")
                        ngrp = 2 * qc + 2          # groups of 2 k-blocks

                        def emit_pv(g, e_t):
                            # PV + Z for group g (runs one group behind the
                            # scores stream so the PE never waits on exp)
                            for j in range(2):
                                kb = 2 * g + j
                                off = (kb - 4 * qc) * 128 if kb >= 4 * qc else 0
                                st = (kb == 0)
                                sp = (kb == 4 * qc + 3)
                                nc.tensor.matmul(
                                    ops_t[:, off:], v_sb[:, kb, kv * HD:(kv + 1) * HD],
                                    e_t[:, j, off:], start=st, stop=sp,
                                    skip_group_check=True)
                                nc.tensor.matmul(
                                    zps_t[0:1, off:], ones_bf[:], e_t[:, j, off:],
                                    start=st, stop=sp, skip_group_check=True)

                        prev_pv = None
                        for g in range(ngrp):
                            diag = (g >= 2 * qc)   # last two groups touch diagonal
                            sps = ps2s.tile([128, 2, QC], F32, tag="sps")
                            e_t = epool.tile([128, 2, QC], BF16, tag="etile")
                            for j in range(2):
                                kb = 2 * g + j
                                # full q-width even on diagonal blocks: the
                                # mask zeroes invalid columns, and a fully
                                # written tile avoids uninit-PSUM reads in exp
                                nc.tensor.matmul(
                                    sps[:, j, :], kt_sb[kv][:, kb * 128:(kb + 1) * 128],
                                    qt_sb[h][:, qsl],
                                    start=True, stop=True)
                            nc.scalar.activation(
                                e_t[:].rearrange("p a b -> p (a b)"),
                                sps[:].rearrange("p a b -> p (a b)"),
                                AF.Exp, scale=SCALE)
                            if diag:
                                gi = g - 2 * qc
                                nc.vector.tensor_tensor(
                                    e_t[:].rearrange("p a b -> p (a b)"),
                                    e_t[:].rearrange("p a b -> p (a b)"),
                                    mask_sb[:, gi, :], OP.mult)
                            if prev_pv is not None:
                                emit_pv(*prev_pv)
                                if g == 1 and pending is not None:
                                    emit_finalize(pending)
                                    pending = None
                            prev_pv = (g, e_t)
                        emit_pv(*prev_pv)

                        # Z drain on DVE (reciprocal chain runs while the
                        # next head's first group streams on PE)
                        z_sb = small.tile([1, QC], F32, tag="zsb")
                        nc.vector.tensor_copy(z_sb[:], zps_t[0:1, :])
                        rz = small.tile([1, QC], F32, tag="rz")
                        nc.vector.reciprocal_approx_fast(rz[:], z_sb[:])
                        rz_r = small.tile([1, QC], F32R, tag="rzr")
                        nc.vector.tensor_copy(rz_r[:], rz[:])
                        pending = (ops_t, rz_r, ot_sb[h], qsl, qc, h)

                        for _ in range(2):
                            if rope_deferred:
                                rope_chain(*rope_deferred.pop(0))

                    # bulk-emit the PREVIOUS qc's projection (its last head
                    # was finalized during this qc's first head)
                    while proj_q:
                        proj_unit(*proj_q.pop(0), ps2s)
                    proj_q = [(sb, dcp) for sb in range(4 * qc, 4 * qc + 4)
                              for dcp in range(2)]
                emit_finalize(pending)
                while proj_q:
                    proj_unit(*proj_q.pop(0), ps2s)

    nc.compile()
    return nc


_NC_CACHE = None


def _get_nc():
    global _NC_CACHE
    if _NC_CACHE is None:
        _NC_CACHE = _build_nc()
    return _NC_CACHE


def _rope_tables():
    inv = 1.0 / (ROPE_THETA ** (np.arange(0, HD, 2, dtype=np.float64) / HD))  # [64]
    t = np.arange(S, dtype=np.float64)
    ang = np.outer(inv, t)                      # [64, S]
    cos = np.cos(ang).astype(np.float32)
    sin = np.sin(ang).astype(np.float32)
    cos128 = np.concatenate([cos, cos], axis=0).astype(BF)  # [128, S]
    sinM = np.concatenate([-sin, sin], axis=0).astype(BF)
    return cos128, sinM


def _masks():
    # dmask[g] covers a group of 2 k-blocks at diagonal offsets (2g*128,(2g+1)*128)
    q = np.arange(QC)
    m = np.zeros((2, 128, 2 * QC), np.float32)
    for g in range(2):
        for j in range(2):
            off = (2 * g + j) * 128
            k = np.arange(128) + off
            m[g, :, j * QC:(j + 1) * QC] = (k[:, None] <= q[None, :])
    return m.astype(BF)


def prepare_inputs(x, wq, wk, wv, wo):
    """Build the 8 per-core input dicts from full inputs."""
    perm = np.concatenate([np.arange(0, HD, 2), np.arange(1, HD, 2)])
    cos128, sinM = _rope_tables()
    dmask = _masks()

    x = np.asarray(x, np.float32)
    wq = np.asarray(wq, np.float32).reshape(HQ, HD, D)[:, perm, :]
    wk = np.asarray(wk, np.float32).reshape(HKV, HD, D)[:, perm, :]
    wv = np.asarray(wv, np.float32).reshape(HKV, HD, D)
    wo = np.asarray(wo, np.float32)              # [D, HQ*HD]

    in_maps = []
    for c in range(NCORES):
        b, hg = divmod(c, TP)
        qh = slice(hg * HQL, (hg + 1) * HQL)
        kh = slice(hg * HKL, (hg + 1) * HKL)
        # p-major: [128, NKT, *] so per-partition DRAM runs are 4KB+
        xT = np.ascontiguousarray(
            x[b].T.reshape(NKT, 128, S).transpose(1, 0, 2)).astype(BF)
        wq_t = np.ascontiguousarray(
            wq[qh].reshape(HQL * HD, D).T.reshape(NKT, 128, HQL * HD)
            .transpose(1, 0, 2)).astype(BF)
        wk_t = np.ascontiguousarray(
            wk[kh].reshape(HKL * HD, D).T.reshape(NKT, 128, HKL * HD)
            .transpose(1, 0, 2)).astype(BF)
        wv_t = np.ascontiguousarray(
            wv[kh].reshape(HKL * HD, D).T.reshape(NKT, 128, HKL * HD)
            .transpose(1, 0, 2)).astype(BF)
        wo_t = np.ascontiguousarray(
            wo[:, hg * HQL * HD:(hg + 1) * HQL * HD].T.reshape(HQL, HD, D)
            .transpose(1, 0, 2)).astype(BF)
        in_maps.append({
            "xT": xT, "wq_t": wq_t, "wk_t": wk_t, "wv_t": wv_t, "wo_t": wo_t,
            "cos128": cos128, "sinM": sinM, "dmask": dmask,
        })
    return in_maps


def _install_ntff_hook():
    """The agent image's antenv lacks axon_hooks; synthesize it so
    run_bass_kernel_spmd(trace=True) can capture NTFF profiles."""
    import sys as _sys
    import types, contextlib, ctypes

    if "antenv.axon_hooks" in _sys.modules:
        return
    so_path = "/opt/axon/libaxon_pjrt.so"
    lib = ctypes.CDLL(so_path)
    if not hasattr(lib, "axon_start_nrt_profile"):
        return
    lib.axon_start_nrt_profile.argtypes = [ctypes.POINTER(ctypes.c_int64),
                                           ctypes.c_size_t]
    lib.axon_start_nrt_profile.restype = ctypes.c_int64
    lib.axon_stop_nrt_profile.argtypes = [ctypes.c_char_p]
    lib.axon_stop_nrt_profile.restype = ctypes.c_int64

    @contextlib.contextmanager
    def _hook(output_dir, device_ids):
        import jax
        jax.devices()
        if device_ids:
            ids = (ctypes.c_int64 * len(device_ids))(*device_ids)
            rc = lib.axon_start_nrt_profile(ids, len(device_ids))
        else:
            rc = lib.axon_start_nrt_profile(None, 0)
        if rc != 0:
            raise RuntimeError(f"axon_start_nrt_profile rc={rc}")
        try:
            yield
        finally:
            n = lib.axon_stop_nrt_profile(str(output_dir).encode())
            print(f"ntff profile: {n} file(s) written to {output_dir}",
                  file=_sys.stderr)

    mod = types.ModuleType("antenv.axon_hooks")
    mod.get_axon_ntff_profile_hook = lambda: _hook
    mod.set_axon_ntff_profile_hook = lambda h: None
    _sys.modules["antenv.axon_hooks"] = mod
    try:
        import antenv
        antenv.axon_hooks = mod
    except ImportError:
        pass


def kernel(x, wq, wk, wv, wo, _trace=False, _trace_cores=None):
    in_maps = prepare_inputs(x, wq, wk, wv, wo)
    if _trace:
        _install_ntff_hook()
    nc = _get_nc()
    res = run_bass_kernel_spmd(
        nc, in_maps, core_ids=list(range(NCORES)),
        trace=_trace, trace_cores=_trace_cores)
    out = np.zeros((B, S, D), np.float32)
    for c in range(NCORES):
        b = c // TP
        out[b] += res.results[c]["out"].reshape(S, D).astype(np.float32)
    kernel.last_results = res
    return out


if __name__ == "__main__":
    rng = np.random.default_rng(0)
    x = rng.standard_normal((B, S, D), dtype=np.float32)
    sc = 1.0 / np.sqrt(D)
    wq = (rng.standard_normal((HQ * HD, D), dtype=np.float32) * sc)
    wk = (rng.standard_normal((HKV * HD, D), dtype=np.float32) * sc)
    wv = (rng.standard_normal((HKV * HD, D), dtype=np.float32) * sc)
    wo = (rng.standard_normal((D, HQ * HD), dtype=np.float32) * sc)
    out = kernel(x, wq, wk, wv, wo)
    print("ran", out.shape, out.dtype, float(np.abs(out).mean()))
